# revision 2
# baseline (speedup 1.0000x reference)
# Trainium2 Bass kernel: 2:4 structured activation pruning + Linear.
#
#   out = magnitude_prune_2of4(x.reshape(-1, 4096)) @ weight.T
#
# Sharding: data-parallel over the flattened token dim (16384 tokens ->
# 2048/core across 8 cores); weight replicated (host-transposed + bf16).
# No collectives.
#
# v2 pipeline (per 128-token tile):
#   DMA x (f32) -> DVE pairwise min/max tree -> exact-f32 per-group-of-4
#   2nd-max threshold -> DVE PRUNE24 select writes bf16 -> DMA XBAR
#   transpose (SBUF->SBUF, 16x128 hw tiles) deposits [d, tok] bf16 chunks
#   -> PE matmul bf16 (1 cyc/row) accumulating 32 d-chunks into PSUM
#   -> ACT PSUM->SBUF copy -> DMA out.
# PE runs ONLY the 1024 N=512 matmuls (~213ns each): the 512 PE transposes
# of the old version moved to the DMA XBAR engines.
import numpy as np

N_CORES = 8
BS, SEQ, D = 4, 4096, 4096
OUTF = 1024
TOK_TOTAL = BS * SEQ
TOK = TOK_TOTAL // N_CORES      # 2048 tokens per core
P = 128                         # SBUF partitions
NT = TOK // P                   # 16 token tiles per core
NCH = D // P                    # 32 d-chunks of 128
HALF = D // 2                   # 2048
NCH_H = NCH // 2                # 16 d-chunks per half

_compiled = None
_custom_ops = None


def _register_custom_dve():
    # Fused DVE ops: pairwise abs-max/abs-min, and the pruning select
    # out = |x| >= thr ? x : 0 (exact f32 compare, bf16 output).
    global _custom_ops
    if _custom_ops is not None:
        return _custom_ops
    from concourse import dve_ops as Dv
    from concourse.dve_spec import Spec, Src0, Src1, Zero, maxx, minn, select, lower
    from concourse.dve_uop import DveOpSpec

    def mk(name, body, reference):
        spec = Spec(body=body, reference=reference)
        shas = {}
        for ver in ("v3", "v4"):
            try:
                u = lower(spec, ver=ver)
                shas[ver] = DveOpSpec(name=name, opcode=1, uops=u,
                                      rd1_en=True).sha(ver)
            except Exception:
                if ver == "v3":
                    raise
        return Dv.DveOp(name=name, spec=spec, subdim=False, uops_sha=shas)

    absa = maxx(Src0, Zero - Src0)
    absb = maxx(Src1, Zero - Src1)
    ops = (
        mk("ABS_MAX2_ANT", maxx(absa, absb),
           lambda in0, in1: np.maximum(np.abs(in0), np.abs(in1))),
        mk("ABS_MIN2_ANT", minn(absa, absb),
           lambda in0, in1: np.minimum(np.abs(in0), np.abs(in1))),
        mk("PRUNE24_ANT", select(maxx(Src0, Zero - Src0) >= Src1, Src0, Zero),
           lambda in0, in1: np.where(np.abs(in0) >= in1, in0, 0.0)),
    )
    for op in ops:
        if op.name not in Dv._SUB_OPCODE_FOR_NAME:
            Dv.OPS.append(op)
            Dv.CUSTOM_DVE_SPECS[op.name] = op.spec
            Dv._SUB_OPCODE_FOR_NAME[op.name] = (
                Dv._CUSTOM_DVE_ROW_BASE + len(Dv._SUB_OPCODE_FOR_NAME))
    _custom_ops = ops
    return ops


def _build():
    import concourse.tile as tile
    import concourse.mybir as mybir
    from concourse import bacc

    ABS_MAX2, ABS_MIN2, PRUNE24 = _register_custom_dve()
    f32 = mybir.dt.float32
    bf16 = mybir.dt.bfloat16
    Alu = mybir.AluOpType

    nc = bacc.Bacc("TRN2", target_bir_lowering=False, debug=False,
                   num_devices=N_CORES)
    xs_ap = nc.dram_tensor("xs", [TOK, D], f32, kind="ExternalInput").ap()
    wt_ap = nc.dram_tensor("wt", [D, OUTF], bf16, kind="ExternalInput").ap()
    o_ap = nc.dram_tensor("o", [TOK, OUTF], f32, kind="ExternalOutput").ap()

    with tile.TileContext(nc) as tc:
        with tc.tile_pool(name="wpool", bufs=1) as wpool, \
             tc.tile_pool(name="xin", bufs=2) as xin, \
             tc.tile_pool(name="mwork", bufs=1) as mwork, \
             tc.tile_pool(name="xsp", bufs=2) as xspp, \
             tc.tile_pool(name="xtp", bufs=2) as xtp, \
             tc.tile_pool(name="outp", bufs=2) as outp, \
             tc.tile_pool(name="pso", bufs=4, space="PSUM") as pso:

            # weight.T resident in SBUF as bf16: [d-in-chunk, chunk, outf].
            # gpsimd (SWDGE) queue so the load doesn't block x-tile loads.
            w_sb = wpool.tile([P, NCH, OUTF], bf16)
            for c in range(NCH):
                nc.gpsimd.dma_start(out=w_sb[:, c, :],
                                    in_=wt_ap[c * P:(c + 1) * P, :])

            for i in range(NT):
                xh = xin.tile([P, D], f32, tag="xh")
                nc.sync.dma_start(out=xh, in_=xs_ap[i * P:(i + 1) * P, :])
                # threshold = 2nd-largest |x| per contiguous group of 4,
                # exact in f32 (tie behavior == host _fix_ties contract)
                x2 = xh.rearrange("p (g two) -> p g two", two=2)
                mx = mwork.tile([P, D // 2], f32, tag="mx")
                mn = mwork.tile([P, D // 2], f32, tag="mn")
                nc.vector._custom_dve(ABS_MAX2, out=mx,
                                      in0=x2[:, :, 0], in1=x2[:, :, 1])
                nc.vector._custom_dve(ABS_MIN2, out=mn,
                                      in0=x2[:, :, 0], in1=x2[:, :, 1])
                # compact in place: writes trail the strided reads
                mx2 = mx.rearrange("p (g two) -> p g two", two=2)
                mn2 = mn.rearrange("p (g two) -> p g two", two=2)
                mm = mx[:, :D // 4]
                nm = mn[:, :D // 4]
                nc.vector.tensor_tensor(mm, mx2[:, :, 0], mx2[:, :, 1], Alu.min)
                nc.vector.tensor_tensor(nm, mn2[:, :, 0], mn2[:, :, 1], Alu.max)
                thr = mm
                nc.vector.tensor_tensor(thr, mm, nm, Alu.max)
                # prune -> bf16, then DMA-XBAR transpose each half so the
                # PE can start on half A while half B still prunes
                xsp = xspp.tile([P, D], bf16, tag="xsp")
                xspT = xtp.tile([P, NCH, P], bf16, tag="xspT")
                for h in range(2):
                    lo = h * HALF
                    g0 = h * (HALF // 4)
                    thr_b = thr[:, g0:g0 + HALF // 4].unsqueeze(2) \
                        .broadcast_to([P, HALF // 4, 4])
                    sp = xsp[:, lo:lo + HALF]
                    nc.vector._custom_dve(
                        PRUNE24,
                        out=sp.rearrange("p (g four) -> p g four", four=4),
                        in0=xh[:, lo:lo + HALF]
                            .rearrange("p (g four) -> p g four", four=4),
                        in1=thr_b)
                    nc.scalar.dma_start(
                        out=xspT[:, h * NCH_H:(h + 1) * NCH_H, :],
                        in_=sp, transpose=True)
                # matmul: psum[tok, outf-half] += xspT[c].T @ wT[c]
                for n in range(2):
                    pout = pso.tile([P, OUTF // 2], f32)
                    for c in range(NCH):
                        nc.tensor.matmul(pout,
                                         xspT[:, c, :],
                                         w_sb[:, c, n * 512:(n + 1) * 512],
                                         start=(c == 0), stop=(c == NCH - 1))
                    osb = outp.tile([P, OUTF // 2], f32)
                    nc.scalar.copy(osb, pout)
                    nc.sync.dma_start(
                        out=o_ap[i * P:(i + 1) * P, n * 512:(n + 1) * 512],
                        in_=osb)
    nc.compile()
    return nc


def _get_compiled():
    global _compiled
    if _compiled is None:
        _compiled = _build()
    return _compiled


def _fix_ties(x_flat):
    # The device keeps elements with |x| >= (2nd-largest |x| of the group).
    # On an exact fp32 tie |2nd|==|3rd| that keeps 3 elements, while the
    # reference (top_k, stable) keeps the lower-indexed 2. Pre-zero the
    # reference-dropped elements of tied groups so the device agrees; the
    # zeroed elements are dropped either way, so values are unaffected.
    g = np.abs(x_flat.reshape(-1, 4))
    m1 = np.maximum(g[:, 0], g[:, 1]); n1 = np.minimum(g[:, 0], g[:, 1])
    m2 = np.maximum(g[:, 2], g[:, 3]); n2 = np.minimum(g[:, 2], g[:, 3])
    thr = np.maximum(np.minimum(m1, m2), np.maximum(n1, n2))
    third = np.minimum(np.minimum(m1, m2), np.maximum(n1, n2))
    tied = np.flatnonzero(thr == third)
    if len(tied) == 0:
        return x_flat
    x_flat = x_flat.copy()
    gv = x_flat.reshape(-1, 4)
    for t in tied:
        row = gv[t]
        order = np.argsort(-np.abs(row), kind="stable")
        row[order[2:]] = 0.0
    return x_flat


def _prepare_in_maps(x, weight):
    import ml_dtypes
    x_flat = np.ascontiguousarray(x.reshape(TOK_TOTAL, D), dtype=np.float32)
    x_flat = _fix_ties(x_flat)
    wt = np.ascontiguousarray(weight.T, dtype=np.float32) \
        .astype(ml_dtypes.bfloat16)
    return [{"xs": x_flat[c * TOK:(c + 1) * TOK], "wt": wt}
            for c in range(N_CORES)]


def kernel(x: np.ndarray, weight: np.ndarray) -> np.ndarray:
    from concourse.bass_utils import run_bass_kernel_spmd

    nc = _get_compiled()
    in_maps = _prepare_in_maps(x, weight)
    res = run_bass_kernel_spmd(nc, in_maps, core_ids=list(range(N_CORES)))
    out = np.concatenate([res.results[c]["o"] for c in range(N_CORES)], axis=0)
    return out.reshape(BS, SEQ, OUTF)


# revision 3
# speedup vs baseline: 1.0336x; 1.0336x over previous
# Trainium2 Bass kernel: 2:4 structured activation pruning + Linear.
#
#   out = magnitude_prune_2of4(x.reshape(-1, 4096)) @ weight.T
#
# Sharding: data-parallel over the flattened token dim (16384 tokens ->
# 2048/core across 8 cores); weight replicated (host-transposed + bf16).
# No collectives.
#
# v2 pipeline (per 128-token tile):
#   DMA x (f32) -> DVE pairwise min/max tree -> exact-f32 per-group-of-4
#   2nd-max threshold -> DVE PRUNE24 select writes bf16 -> DMA XBAR
#   transpose (SBUF->SBUF, 16x128 hw tiles) deposits [d, tok] bf16 chunks
#   -> PE matmul bf16 (1 cyc/row) accumulating 32 d-chunks into PSUM
#   -> ACT PSUM->SBUF copy -> DMA out.
# PE runs ONLY the 1024 N=512 matmuls (~213ns each): the 512 PE transposes
# of the old version moved to the DMA XBAR engines.
import numpy as np

N_CORES = 8
BS, SEQ, D = 4, 4096, 4096
OUTF = 1024
TOK_TOTAL = BS * SEQ
TOK = TOK_TOTAL // N_CORES      # 2048 tokens per core
P = 128                         # SBUF partitions
NT = TOK // P                   # 16 token tiles per core
NCH = D // P                    # 32 d-chunks of 128
HALF = D // 2                   # 2048
NCH_H = NCH // 2                # 16 d-chunks per half

_compiled = None
_custom_ops = None


def _register_custom_dve():
    # Fused DVE ops: pairwise abs-max/abs-min, and the pruning select
    # out = |x| >= thr ? x : 0 (exact f32 compare, bf16 output).
    global _custom_ops
    if _custom_ops is not None:
        return _custom_ops
    from concourse import dve_ops as Dv
    from concourse.dve_spec import Spec, Src0, Src1, Zero, maxx, minn, select, lower
    from concourse.dve_uop import DveOpSpec

    def mk(name, body, reference):
        spec = Spec(body=body, reference=reference)
        shas = {}
        for ver in ("v3", "v4"):
            try:
                u = lower(spec, ver=ver)
                shas[ver] = DveOpSpec(name=name, opcode=1, uops=u,
                                      rd1_en=True).sha(ver)
            except Exception:
                if ver == "v3":
                    raise
        return Dv.DveOp(name=name, spec=spec, subdim=False, uops_sha=shas)

    absa = maxx(Src0, Zero - Src0)
    absb = maxx(Src1, Zero - Src1)
    ops = (
        mk("ABS_MAX2_ANT", maxx(absa, absb),
           lambda in0, in1: np.maximum(np.abs(in0), np.abs(in1))),
        mk("ABS_MIN2_ANT", minn(absa, absb),
           lambda in0, in1: np.minimum(np.abs(in0), np.abs(in1))),
        mk("PRUNE24_ANT", select(maxx(Src0, Zero - Src0) >= Src1, Src0, Zero),
           lambda in0, in1: np.where(np.abs(in0) >= in1, in0, 0.0)),
    )
    for op in ops:
        if op.name not in Dv._SUB_OPCODE_FOR_NAME:
            Dv.OPS.append(op)
            Dv.CUSTOM_DVE_SPECS[op.name] = op.spec
            Dv._SUB_OPCODE_FOR_NAME[op.name] = (
                Dv._CUSTOM_DVE_ROW_BASE + len(Dv._SUB_OPCODE_FOR_NAME))
    _custom_ops = ops
    return ops


def _build():
    import concourse.tile as tile
    import concourse.mybir as mybir
    from concourse import bacc

    ABS_MAX2, ABS_MIN2, PRUNE24 = _register_custom_dve()
    f32 = mybir.dt.float32
    bf16 = mybir.dt.bfloat16
    Alu = mybir.AluOpType

    nc = bacc.Bacc("TRN2", target_bir_lowering=False, debug=False,
                   num_devices=N_CORES)
    xs_ap = nc.dram_tensor("xs", [TOK, D], f32, kind="ExternalInput").ap()
    wt_ap = nc.dram_tensor("wt", [D, OUTF], bf16, kind="ExternalInput").ap()
    o_ap = nc.dram_tensor("o", [TOK, OUTF], f32, kind="ExternalOutput").ap()

    with tile.TileContext(nc) as tc:
        with tc.tile_pool(name="wpool", bufs=1) as wpool, \
             tc.tile_pool(name="xin", bufs=2) as xin, \
             tc.tile_pool(name="mwork", bufs=1) as mwork, \
             tc.tile_pool(name="xsp", bufs=2) as xspp, \
             tc.tile_pool(name="xtp", bufs=2) as xtp, \
             tc.tile_pool(name="outp", bufs=4) as outp, \
             tc.tile_pool(name="pso", bufs=4, space="PSUM") as pso:

            # weight.T resident in SBUF as bf16: [d-in-chunk, chunk, outf].
            # gpsimd (SWDGE) queue so the load doesn't block x-tile loads.
            w_sb = wpool.tile([P, NCH, OUTF], bf16)
            for c in range(NCH):
                nc.gpsimd.dma_start(out=w_sb[:, c, :],
                                    in_=wt_ap[c * P:(c + 1) * P, :])

            def front(i, spans):
                # x load, 2:4 threshold (exact f32), prune->bf16, XBAR
                # transpose. sync queue: x-in + transposes (their readiness
                # order matches FIFO order); DVE: all the math.
                xh = xin.tile([P, D], f32, tag="xh")
                xsp = xspp.tile([P, D], bf16, tag="xsp")
                xspT = xtp.tile([P, NCH, P], bf16, tag="xspT")
                for lo, w in spans:
                    nc.sync.dma_start(out=xh[:, lo:lo + w],
                                      in_=xs_ap[i * P:(i + 1) * P,
                                                lo:lo + w])
                for lo, w in spans:
                    xv = xh[:, lo:lo + w]
                    x2 = xv.rearrange("p (g two) -> p g two", two=2)
                    mx = mwork.tile([P, D // 2], f32, tag="mx")
                    mn = mwork.tile([P, D // 2], f32, tag="mn")
                    mxs = mx[:, lo // 2:(lo + w) // 2]
                    mns = mn[:, lo // 2:(lo + w) // 2]
                    nc.vector._custom_dve(ABS_MAX2, out=mxs,
                                          in0=x2[:, :, 0], in1=x2[:, :, 1])
                    nc.vector._custom_dve(ABS_MIN2, out=mns,
                                          in0=x2[:, :, 0], in1=x2[:, :, 1])
                    # compact in place: writes trail the strided reads
                    mx2 = mxs.rearrange("p (g two) -> p g two", two=2)
                    mn2 = mns.rearrange("p (g two) -> p g two", two=2)
                    mm = mxs[:, :w // 4]
                    nm = mns[:, :w // 4]
                    nc.vector.tensor_tensor(mm, mx2[:, :, 0], mx2[:, :, 1],
                                            Alu.min)
                    nc.vector.tensor_tensor(nm, mn2[:, :, 0], mn2[:, :, 1],
                                            Alu.max)
                    thr = mm
                    nc.vector.tensor_tensor(thr, mm, nm, Alu.max)
                    thr_b = thr.unsqueeze(2).broadcast_to([P, w // 4, 4])
                    sp = xsp[:, lo:lo + w]
                    nc.vector._custom_dve(
                        PRUNE24,
                        out=sp.rearrange("p (g four) -> p g four", four=4),
                        in0=xv.rearrange("p (g four) -> p g four", four=4),
                        in1=thr_b)
                    nc.sync.dma_start(
                        out=xspT[:, lo // P:(lo + w) // P, :],
                        in_=sp, transpose=True)
                return xspT

            def back(i, xspT):
                # matmuls on PE; PSUM->SBUF copy + out DMA on scalar queue
                for n in range(2):
                    pout = pso.tile([P, OUTF // 2], f32)
                    for c in range(NCH):
                        nc.tensor.matmul(pout,
                                         xspT[:, c, :],
                                         w_sb[:, c, n * 512:(n + 1) * 512],
                                         start=(c == 0), stop=(c == NCH - 1))
                    osb = outp.tile([P, OUTF // 2], f32)
                    nc.scalar.copy(osb, pout)
                    nc.scalar.dma_start(
                        out=o_ap[i * P:(i + 1) * P, n * 512:(n + 1) * 512],
                        in_=osb)

            # software pipeline: emit tile i+1's front before tile i's
            # matmul stage so every engine's FIFO order matches readiness
            # order (no head-of-line blocking behind matmul-dependent ops).
            halves = [(0, HALF), (HALF, D)]
            halves = [(lo, hi - lo) for lo, hi in halves]
            prev = front(0, halves)          # tile 0 per-half: shorter fill
            for i in range(1, NT):
                cur = front(i, [(0, D)])
                back(i - 1, prev)
                prev = cur
            back(NT - 1, prev)
    nc.compile()
    return nc


def _get_compiled():
    global _compiled
    if _compiled is None:
        _compiled = _build()
    return _compiled


def _fix_ties(x_flat):
    # The device keeps elements with |x| >= (2nd-largest |x| of the group).
    # On an exact fp32 tie |2nd|==|3rd| that keeps 3 elements, while the
    # reference (top_k, stable) keeps the lower-indexed 2. Pre-zero the
    # reference-dropped elements of tied groups so the device agrees; the
    # zeroed elements are dropped either way, so values are unaffected.
    g = np.abs(x_flat.reshape(-1, 4))
    m1 = np.maximum(g[:, 0], g[:, 1]); n1 = np.minimum(g[:, 0], g[:, 1])
    m2 = np.maximum(g[:, 2], g[:, 3]); n2 = np.minimum(g[:, 2], g[:, 3])
    thr = np.maximum(np.minimum(m1, m2), np.maximum(n1, n2))
    third = np.minimum(np.minimum(m1, m2), np.maximum(n1, n2))
    tied = np.flatnonzero(thr == third)
    if len(tied) == 0:
        return x_flat
    x_flat = x_flat.copy()
    gv = x_flat.reshape(-1, 4)
    for t in tied:
        row = gv[t]
        order = np.argsort(-np.abs(row), kind="stable")
        row[order[2:]] = 0.0
    return x_flat


def _prepare_in_maps(x, weight):
    import ml_dtypes
    x_flat = np.ascontiguousarray(x.reshape(TOK_TOTAL, D), dtype=np.float32)
    x_flat = _fix_ties(x_flat)
    wt = np.ascontiguousarray(weight.T, dtype=np.float32) \
        .astype(ml_dtypes.bfloat16)
    return [{"xs": x_flat[c * TOK:(c + 1) * TOK], "wt": wt}
            for c in range(N_CORES)]


def kernel(x: np.ndarray, weight: np.ndarray) -> np.ndarray:
    from concourse.bass_utils import run_bass_kernel_spmd

    nc = _get_compiled()
    in_maps = _prepare_in_maps(x, weight)
    res = run_bass_kernel_spmd(nc, in_maps, core_ids=list(range(N_CORES)))
    out = np.concatenate([res.results[c]["o"] for c in range(N_CORES)], axis=0)
    return out.reshape(BS, SEQ, OUTF)


# revision 6
# speedup vs baseline: 1.2215x; 1.1818x over previous
# Trainium2 Bass kernel: 2:4 structured activation pruning + Linear.
#
#   out = magnitude_prune_2of4(x.reshape(-1, 4096)) @ weight.T
#
# Sharding: data-parallel over the flattened token dim (16384 tokens ->
# 2048/core across 8 cores); weight replicated (host-transposed + bf16).
# No collectives.
#
# v4 pipeline (per 128-token tile):
#   DMA x (bf16, host-cast; host pre-zeroes bf16-tie-ambiguous groups so
#   device top-2 selection matches the reference exactly) -> DVE pairwise
#   min/max tree in bf16 with contiguous-pair addressing (2x DVE modes)
#   -> PRUNE24 select -> DMA XBAR transpose (SBUF->SBUF) -> PE matmul
#   bf16 accumulating 32 d-chunks into PSUM, c-outer loop so one
#   stationary load feeds both output halves -> ACT PSUM->SBUF -> DMA out.
import numpy as np

N_CORES = 8
BS, SEQ, D = 4, 4096, 4096
OUTF = 1024
TOK_TOTAL = BS * SEQ
TOK = TOK_TOTAL // N_CORES      # 2048 tokens per core
P = 128                         # SBUF partitions
NT = TOK // P                   # 16 token tiles per core
NCH = D // P                    # 32 d-chunks of 128
HALF = D // 2                   # 2048

_compiled = None
_custom_ops = None


def _register_custom_dve():
    # Fused DVE ops: pairwise abs-max/abs-min, and the pruning select
    # out = |x| >= thr ? x : 0.
    global _custom_ops
    if _custom_ops is not None:
        return _custom_ops
    from concourse import dve_ops as Dv
    from concourse.dve_spec import Spec, Src0, Src1, Zero, maxx, minn, select, lower
    from concourse.dve_uop import DveOpSpec

    def mk(name, body, reference):
        spec = Spec(body=body, reference=reference)
        shas = {}
        for ver in ("v3", "v4"):
            try:
                u = lower(spec, ver=ver)
                shas[ver] = DveOpSpec(name=name, opcode=1, uops=u,
                                      rd1_en=True).sha(ver)
            except Exception:
                if ver == "v3":
                    raise
        return Dv.DveOp(name=name, spec=spec, subdim=False, uops_sha=shas)

    absa = maxx(Src0, Zero - Src0)
    absb = maxx(Src1, Zero - Src1)
    ops = (
        mk("ABS_MAX2_ANT", maxx(absa, absb),
           lambda in0, in1: np.maximum(np.abs(in0), np.abs(in1))),
        mk("ABS_MIN2_ANT", minn(absa, absb),
           lambda in0, in1: np.minimum(np.abs(in0), np.abs(in1))),
        mk("PRUNE24_ANT", select(maxx(Src0, Zero - Src0) >= Src1, Src0, Zero),
           lambda in0, in1: np.where(np.abs(in0) >= in1, in0, 0.0)),
    )
    for op in ops:
        if op.name not in Dv._SUB_OPCODE_FOR_NAME:
            Dv.OPS.append(op)
            Dv.CUSTOM_DVE_SPECS[op.name] = op.spec
            Dv._SUB_OPCODE_FOR_NAME[op.name] = (
                Dv._CUSTOM_DVE_ROW_BASE + len(Dv._SUB_OPCODE_FOR_NAME))
    _custom_ops = ops
    return ops


def _build():
    import concourse.tile as tile
    import concourse.mybir as mybir
    from concourse import bacc

    ABS_MAX2, ABS_MIN2, PRUNE24 = _register_custom_dve()
    f32 = mybir.dt.float32
    bf16 = mybir.dt.bfloat16
    Alu = mybir.AluOpType

    nc = bacc.Bacc("TRN2", target_bir_lowering=False, debug=False,
                   num_devices=N_CORES)
    xs_ap = nc.dram_tensor("xs", [TOK, D], bf16, kind="ExternalInput").ap()
    wt_ap = nc.dram_tensor("wt", [D, OUTF], bf16, kind="ExternalInput").ap()
    o_ap = nc.dram_tensor("o", [TOK, OUTF], f32, kind="ExternalOutput").ap()

    with tile.TileContext(nc) as tc:
        with tc.tile_pool(name="wpool", bufs=1) as wpool, \
             tc.tile_pool(name="xin", bufs=2) as xin, \
             tc.tile_pool(name="mwork", bufs=1) as mwork, \
             tc.tile_pool(name="xsp", bufs=2) as xspp, \
             tc.tile_pool(name="xtp", bufs=2) as xtp, \
             tc.tile_pool(name="outp", bufs=4) as outp, \
             tc.tile_pool(name="pso", bufs=4, space="PSUM") as pso:

            # weight.T resident in SBUF as bf16: [d-in-chunk, chunk, outf].
            # gpsimd (SWDGE) queue so the load doesn't block x-tile loads.
            w_sb = wpool.tile([P, NCH, OUTF], bf16)
            for c in range(NCH):
                nc.gpsimd.dma_start(out=w_sb[:, c, :],
                                    in_=wt_ap[c * P:(c + 1) * P, :])

            def front(i, spans):
                # x load, 2:4 threshold (bf16, host-consistent tie fix),
                # prune, XBAR transpose. sync queue: x-in + transposes.
                xh = xin.tile([P, D], bf16, tag="xh")
                xsp = xspp.tile([P, D], bf16, tag="xsp")
                xspT = xtp.tile([P, NCH, P], bf16, tag="xspT")
                for lo, w in spans:
                    nc.sync.dma_start(out=xh[:, lo:lo + w],
                                      in_=xs_ap[i * P:(i + 1) * P,
                                                lo:lo + w])
                for lo, w in spans:
                    ng = w // 4
                    xv = xh[:, lo:lo + w]
                    # pairs (a,b) vs (c,d): both operands contiguous runs
                    # of 2 bf16 so the DVE can pick a packed 2x mode
                    xg = xv.rearrange("p (g h two) -> p g h two",
                                      h=2, two=2)
                    mx = mwork.tile([P, HALF], bf16, tag="mx")
                    mn = mwork.tile([P, HALF], bf16, tag="mn")
                    mxs = mx[:, :w // 2]
                    mns = mn[:, :w // 2]
                    mx_t = mxs.rearrange("p (g two) -> p g two", two=2)
                    mn_t = mns.rearrange("p (g two) -> p g two", two=2)
                    nc.vector._custom_dve(ABS_MAX2, out=mx_t,
                                          in0=xg[:, :, 0, :],
                                          in1=xg[:, :, 1, :])
                    nc.vector._custom_dve(ABS_MIN2, out=mn_t,
                                          in0=xg[:, :, 0, :],
                                          in1=xg[:, :, 1, :])
                    # compact in place: writes trail the strided reads
                    mm = mxs[:, :ng]
                    nm = mns[:, :ng]
                    nc.vector.tensor_tensor(mm, mx_t[:, :, 0], mx_t[:, :, 1],
                                            Alu.min)
                    nc.vector.tensor_tensor(nm, mn_t[:, :, 0], mn_t[:, :, 1],
                                            Alu.max)
                    thr = mm
                    nc.vector.tensor_tensor(thr, mm, nm, Alu.max)
                    thr_b = thr.unsqueeze(2).broadcast_to([P, ng, 4])
                    sp = xsp[:, lo:lo + w]
                    nc.vector._custom_dve(
                        PRUNE24,
                        out=sp.rearrange("p (g four) -> p g four", four=4),
                        in0=xv.rearrange("p (g four) -> p g four", four=4),
                        in1=thr_b)
                    nc.sync.dma_start(
                        out=xspT[:, lo // P:(lo + w) // P, :],
                        in_=sp, transpose=True)
                return xspT

            def back(i, xspT):
                # matmuls on PE: c-outer so each stationary xspT chunk is
                # loaded once and streamed against both outf halves.
                # PSUM->SBUF copy + out DMA on scalar queue.
                pout0 = pso.tile([P, OUTF // 2], f32, tag="pout0")
                pout1 = pso.tile([P, OUTF // 2], f32, tag="pout1")
                pouts = [pout0, pout1]
                for c in range(NCH):
                    for n in range(2):
                        nc.tensor.matmul(pouts[n],
                                         xspT[:, c, :],
                                         w_sb[:, c, n * 512:(n + 1) * 512],
                                         start=(c == 0), stop=(c == NCH - 1))
                for n in range(2):
                    osb = outp.tile([P, OUTF // 2], f32)
                    nc.scalar.copy(osb, pouts[n])
                    nc.scalar.dma_start(
                        out=o_ap[i * P:(i + 1) * P, n * 512:(n + 1) * 512],
                        in_=osb)

            # software pipeline: emit tile i+1's front before tile i's
            # matmul stage so every engine's FIFO order matches readiness
            # order (no head-of-line blocking behind matmul-dependent ops).
            halves = [(0, HALF), (HALF, HALF)]
            prev = front(0, halves)          # tile 0 per-half: shorter fill
            for i in range(1, NT):
                cur = front(i, [(0, D)])
                back(i - 1, prev)
                prev = cur
            back(NT - 1, prev)
    nc.compile()
    return nc


def _get_compiled():
    global _compiled
    if _compiled is None:
        _compiled = _build()
    return _compiled


def _fix_ties_bf16(x_flat):
    # Device selection: keep x_i iff bf16|x_i| >= (2nd-largest bf16|x| of
    # the group). bf16 rounding is monotone, so for groups whose 2nd and
    # 3rd bf16 magnitudes differ the kept SET equals the reference's
    # (top-2 by exact |x|, stable). For ambiguous groups (bf16 2nd == 3rd)
    # pre-zero the two reference-dropped elements: the device then sees
    # them as 0 and keeps exactly the reference pair. Zeroed elements are
    # dropped by the reference anyway, so values are unaffected.
    import ml_dtypes
    xb = x_flat.astype(ml_dtypes.bfloat16)
    b = np.abs(xb.astype(np.float32)).reshape(-1, 4)
    m1 = np.maximum(b[:, 0], b[:, 1]); n1 = np.minimum(b[:, 0], b[:, 1])
    m2 = np.maximum(b[:, 2], b[:, 3]); n2 = np.minimum(b[:, 2], b[:, 3])
    lo_hi = np.minimum(m1, m2); hi_lo = np.maximum(n1, n2)
    second = np.maximum(lo_hi, hi_lo)
    third = np.minimum(lo_hi, hi_lo)
    amb = np.flatnonzero(second == third)
    if len(amb):
        ge = np.abs(x_flat.reshape(-1, 4)[amb])
        order = np.argsort(-ge, axis=1, kind="stable")       # exact, stable
        gb = xb.reshape(-1, 4)
        rows = gb[amb]
        np.put_along_axis(rows, order[:, 2:], 0, axis=1)
        gb[amb] = rows
    return xb


def _prepare_in_maps(x, weight):
    import ml_dtypes
    x_flat = np.ascontiguousarray(x.reshape(TOK_TOTAL, D), dtype=np.float32)
    xb = _fix_ties_bf16(x_flat)
    wt = np.ascontiguousarray(weight.T, dtype=np.float32) \
        .astype(ml_dtypes.bfloat16)
    return [{"xs": xb[c * TOK:(c + 1) * TOK], "wt": wt}
            for c in range(N_CORES)]


def kernel(x: np.ndarray, weight: np.ndarray) -> np.ndarray:
    from concourse.bass_utils import run_bass_kernel_spmd

    nc = _get_compiled()
    in_maps = _prepare_in_maps(x, weight)
    res = run_bass_kernel_spmd(nc, in_maps, core_ids=list(range(N_CORES)))
    out = np.concatenate([res.results[c]["o"] for c in range(N_CORES)], axis=0)
    return out.reshape(BS, SEQ, OUTF)
